# revision 98
# baseline (speedup 1.0000x reference)
"""DropBlock kernel for Trainium2, 8 NeuronCores, batch-sharded data parallel.

Reference computation (B,C,H,W = 128,64,56,56, block=5, gamma=0.02):
    mask    = (noise < gamma)                       # (B,C,52,52) corner drops
    dilated = maxpool5x5_full_pad(mask)             # (B,C,56,56)
    block_mask = 1 - dilated
    out = block_mask * x * (numel / sum(block_mask))

Kernel formulation (exact):
    DVE rows:  dmin = min_{5x5}(noise - gamma_lo)   # bf16 log-step min-pool
               mask = sigmoid(2^40 * dmin)          # exact 0/1: the 2^40
                   scaling is exact and |dmin| >= ~9.3e-10 saturates it
    Pool rows: m = sigmoid(2^40*(gamma_lo - noise)) # 0/1 corner indicators
               cnt5x5 = sumpool_{5x5}(m)            # fp8 log-step ADD pool
               mask = sigmoid(2^39 - 2^40*cnt5x5)   # == (cnt5x5 == 0)
    count = fused f32 accum of the mask sigmoids on ACT, AllReduce.

The two formulations are exactly equivalent; the Pool engine's ucode
supports only tensor add/copy (no min/max/mult — those fail the backend
engine check), so its rows use the additive form on exact 0/1 values.
fp8 partial sums up to 25 can round (>16) but never to zero, so the
(cnt5x5 == 0) test stays exact.

Each core: 16 batches x 64 ch = 1024 images -> 8 tiles of 128 images
(images on partitions, image pixels along the free dimension).

Schedule: per tile, Pool owns output rows [0, S), DVE rows [S, 56);
the chains share no buffers and each instruction carries at most one
cross-engine wait.  ACT runs the subtract / indicator / mask sigmoids,
emitted two tiles ahead (subtract+indicator) and one behind (mask) so
the in-order ACT queue never gates the chains.  Phase 2 (x*scale*mask)
runs on DVE alone, paced by the store stream.  DMA floor: 36.8 MB /
360 GBps = 102 us.
"""

import sys

sys.path.insert(0, "/opt/trn_rl_repo")

import numpy as np

import concourse.bacc as bacc
import concourse.bass as bass
import concourse.tile as tile
import concourse.mybir as mybir
from concourse import bass_isa
from concourse.bass_utils import run_bass_kernel_spmd

N_CORES = 8
B, C, H, W = 128, 64, 56, 56
BLK = 5
GAMMA = 0.02
NH, NW = H - (BLK - 1), W - (BLK - 1)  # 52, 52 noise dims
B_SH = B // N_CORES  # 16 batches per core
IMGS = B_SH * C  # 1024 images per core
P = 128  # partitions
NTILES = IMGS // P  # 8 tiles per core
NPIX = NH * NW  # 2704 noise pixels/image
OPIX = H * W  # 3136 out pixels/image
VPW = NW + 2 * (BLK - 1)  # 60 cols in padded horizontal buffer (4+52+4)
COUNT_M = float(B * C * H * W)  # 25690112.0

# Largest f32 strictly below 0.02f: keep <=> noise >= 0.02f.
GAMMA_LO = float(np.nextafter(np.float32(GAMMA), np.float32(0)))
SIG_SCALE = float(2.0 ** 40)
SIG_BIAS = float(SIG_SCALE * np.float32(GAMMA_LO))  # for the indicator
HALF_BIAS = float(2.0 ** 39)  # for the (cnt == 0) test

F32 = mybir.dt.float32
BF16 = mybir.dt.bfloat16
MIN = mybir.AluOpType.min
ADD = mybir.AluOpType.add
MULT = mybir.AluOpType.mult
FP8 = mybir.dt.float8e4
SIGMOID = mybir.ActivationFunctionType.Sigmoid
IDENT = mybir.ActivationFunctionType.Identity

X_PREFETCH = 8
S = 9  # Pool engine owns output rows [0, S); DVE owns [S, 56)
DR = H - S  # DVE rows

_CACHE = {}


def _build(single_core=False, repeat=1, no_cc=False):
    """Build + compile the SPMD bass module once.

    single_core=True builds a collective-free variant (the per-core count is
    used directly as the global count) for cost-model simulation only.
    repeat>1 unrolls the whole pipeline k times (benchmarking only).
    no_cc=True skips the AllReduce on the 8-core build (timing probe only —
    results are wrong by the per-core/global count ratio).
    """
    nc = bacc.Bacc("TRN2", target_bir_lowering=False, debug=False,
                   num_devices=1 if single_core else N_CORES)
    noise_ap = nc.dram_tensor("noise", [IMGS, NPIX], F32,
                              kind="ExternalInput").ap()
    x_ap = nc.dram_tensor("x", [IMGS, OPIX], F32, kind="ExternalInput").ap()
    out_ap = nc.dram_tensor("out", [IMGS, OPIX], F32,
                            kind="ExternalOutput").ap()

    with tile.TileContext(nc) as tc:
        with (
            tc.tile_pool(name="buf", bufs=1) as bp,
            tc.tile_pool(name="xio", bufs=X_PREFETCH) as x_pool,
            tc.tile_pool(name="dram", bufs=1, space="DRAM") as dram_pool,
        ):
            bufs = {}
            # noise triple-buffer: the look-ahead ACT ops (subtract and
            # indicator, two tiles ahead) and the Pool chain (reads its
            # rows until mid-tile) all consume it.
            bufs["nraw"] = [bp.tile([P, NPIX], F32, name=f"nraw{i}")
                            for i in range(3)]
            # DVE-private subtracted rows: tb rows [S, 60) (tb[r] is noise
            # row r-4 shifted; rows 56-59 are persistent 1.0 pads).
            bufs["tbD"] = [bp.tile([P, (60 - S) * NW], BF16,
                                   name=f"tbD{i}") for i in range(2)]
            bufs["aD"] = bp.tile([P, (58 - S) * NW], BF16, name="aD")
            bufs["btD"] = bp.tile([P, DR * NW], BF16, name="btD")
            bufs["vpD"] = bp.tile([P, DR * VPW], BF16, name="vpD")
            bufs["a2D"] = bp.tile([P, DR * (VPW - 2)], BF16, name="a2D")
            bufs["b2D"] = bp.tile([P, DR * W], BF16, name="b2D")
            bufs["dmD"] = [bp.tile([P, DR * W], BF16, name=f"dmD{i}")
                           for i in range(2)]
            # Pool-side fp8 indicator sum-pool scratch:
            # mP: 0/1 corner indicators for noise rows [0, S)
            # a+[k] = m[k] + m[k+1]; c+[k] = a+[k] + m[k+2] (3-row sum)
            bufs["mP"] = [bp.tile([P, S * NW], FP8, name=f"mP{i}")
                          for i in range(2)]
            bufs["aP"] = bp.tile([P, (S - 2) * NW], FP8, name="aP")
            bufs["bP"] = bp.tile([P, (S - 2) * NW], FP8, name="bP")
            bufs["vpP"] = bp.tile([P, S * VPW], FP8, name="vpP")
            bufs["a2P"] = bp.tile([P, S * (VPW - 2)], FP8, name="a2P")
            bufs["b2P"] = bp.tile([P, S * W], FP8, name="b2P")
            bufs["dmP"] = [bp.tile([P, S * W], FP8, name=f"dmP{i}")
                           for i in range(2)]
            bufs["mask"] = bp.tile([P, NTILES * OPIX], FP8, name="mask")
            # two accum slots per tile (Pool-half and DVE-half sigmoid);
            # one extra for the last tile's split DVE sigmoid
            bufs["partials"] = bp.tile([P, 2 * NTILES + 1], F32,
                                       name="partials")
            gbias = bp.tile([P, 1], F32, name="gbias")
            bufs["gbias"] = gbias
            nc.vector.memset(gbias[:], -GAMMA_LO)
            sgbias = bp.tile([P, 1], F32, name="sgbias")
            bufs["sgbias"] = sgbias
            nc.vector.memset(sgbias[:], SIG_BIAS)
            hbias = bp.tile([P, 1], F32, name="hbias")
            bufs["hbias"] = hbias
            nc.vector.memset(hbias[:], HALF_BIAS)

            # one-time pads (persist across tiles and reps)
            for tbD in bufs["tbD"]:
                nc.vector.memset(tbD[:, (56 - S) * NW:(60 - S) * NW], 1.0)
            nc.vector.memset(bufs["aD"][:, (56 - S) * NW:(58 - S) * NW],
                             1.0)
            vpD3 = bufs["vpD"][:].rearrange("p (h w) -> p h w", w=VPW)
            nc.vector.memset(vpD3[:, :, 0:BLK - 1], 1.0)
            nc.vector.memset(vpD3[:, :, W:VPW], 1.0)
            # additive pads are 0
            vpP3 = bufs["vpP"][:].rearrange("p (h w) -> p h w", w=VPW)
            nc.gpsimd.memset(vpP3[:, :, 0:BLK - 1], 0.0)
            nc.gpsimd.memset(vpP3[:, :, W:VPW], 0.0)

            # warm the ACT function tables on a 1-element tile during the
            # DMA lead-in; otherwise LoadActFuncSet (~1.3us) lands in front
            # of the first real subtract and stalls the pipeline.
            warm = bp.tile([P, 1], F32, name="warm")
            nc.scalar.activation(warm[:], gbias[:], IDENT,
                                 bias=gbias[:, 0:1])
            nc.scalar.activation(warm[:], warm[:], SIGMOID)

            for rep in range(repeat):
                _emit_once(nc, tc, noise_ap, x_ap, out_ap, bufs, bp,
                           x_pool, dram_pool, single_core or no_cc, rep)

    nc.compile()
    return nc


def _emit_once(nc, tc, noise_ap, x_ap, out_ap, bufs, bp, x_pool, dram_pool,
               single_core, rep):
    mask_store = bufs["mask"]
    partials = bufs["partials"]
    gbias = bufs["gbias"]
    aP, bP = bufs["aP"], bufs["bP"]
    vpP, a2P, b2P = bufs["vpP"], bufs["a2P"], bufs["b2P"]
    aD, btD = bufs["aD"], bufs["btD"]
    vpD, a2D, b2D = bufs["vpD"], bufs["a2D"], bufs["b2D"]
    vpP3 = vpP[:].rearrange("p (h w) -> p h w", w=VPW)
    a2P3 = a2P[:].rearrange("p (h w) -> p h w", w=VPW - 2)
    b2P3 = b2P[:].rearrange("p (h w) -> p h w", w=W)
    aP3 = aP[:].rearrange("p (h w) -> p h w", w=NW)
    bP3 = bP[:].rearrange("p (h w) -> p h w", w=NW)
    vpD3 = vpD[:].rearrange("p (h w) -> p h w", w=VPW)
    a2D3 = a2D[:].rearrange("p (h w) -> p h w", w=VPW - 2)
    b2D3 = b2D[:].rearrange("p (h w) -> p h w", w=W)

    def noise_dma(t):
        nraw = bufs["nraw"][t % 3]
        if t == 0:
            # halve the cold-start DMA latency
            nc.sync.dma_start(nraw[:, 0:NPIX // 2],
                              noise_ap[bass.ts(t, P), 0:NPIX // 2])
            nc.sync.dma_start(nraw[:, NPIX // 2:NPIX],
                              noise_ap[bass.ts(t, P), NPIX // 2:NPIX])
        else:
            nc.sync.dma_start(nraw[:], noise_ap[bass.ts(t, P), :])

    def sub(t):
        # ACT look-ahead pair for tile t: the DVE-half subtract (tb rows
        # [S,56) = noise rows [S-4,52) - gamma_lo, bf16, sign-exact) and
        # the Pool-half 0/1 indicators m = (noise rows [0,S) < gamma).
        nraw = bufs["nraw"][t % 3]
        tbD = bufs["tbD"][t % 2]
        mP = bufs["mP"][t % 2]
        nc.scalar.activation(mP[:], nraw[:, 0:S * NW], SIGMOID,
                             scale=-SIG_SCALE, bias=bufs["sgbias"][:, 0:1])
        if t == 0:
            nc.scalar.activation(tbD[:, 0:(30 - S) * NW],
                                 nraw[:, (S - 4) * NW:NPIX // 2], IDENT,
                                 bias=gbias[:, 0:1])
            nc.scalar.activation(tbD[:, (30 - S) * NW:(56 - S) * NW],
                                 nraw[:, NPIX // 2:NPIX], IDENT,
                                 bias=gbias[:, 0:1])
        else:
            nc.scalar.activation(tbD[:, 0:(56 - S) * NW],
                                 nraw[:, (S - 4) * NW:NPIX], IDENT,
                                 bias=gbias[:, 0:1])

    def sigmoid(t):
        # block mask as exact 0/1 with the per-tile count accumulated in
        # f32 on ACT; one op per engine-half, one tile behind the chain.
        # Pool half: mask = (cnt5x5 == 0) via sigmoid(2^39 - 2^40*cnt).
        # DVE half: mask = (dmin > 0) via sigmoid(2^40*dmin).
        mslice = mask_store[:, t * OPIX:(t + 1) * OPIX]
        nc.scalar.activation(
            mslice[:, 0:S * W], bufs["dmP"][t % 2][:], SIGMOID,
            scale=-SIG_SCALE, bias=bufs["hbias"][:, 0:1],
            accum_out=partials[:, 2 * t:2 * t + 1])
        if t == NTILES - 1:
            # the last tile's DVE sigmoid is on the count's critical path:
            # its dm (and this sigmoid) are split in half so only the
            # second half's latency sits after the chain's end.
            R1 = DR // 2
            nc.scalar.activation(
                mslice[:, S * W:(S + R1) * W],
                bufs["dmD"][t % 2][:, 0:R1 * W], SIGMOID, scale=SIG_SCALE,
                accum_out=partials[:, 2 * t + 1:2 * t + 2])
            nc.scalar.activation(
                mslice[:, (S + R1) * W:OPIX],
                bufs["dmD"][t % 2][:, R1 * W:DR * W], SIGMOID,
                scale=SIG_SCALE,
                accum_out=partials[:, 2 * t + 2:2 * t + 3])
        else:
            nc.scalar.activation(
                mslice[:, S * W:OPIX], bufs["dmD"][t % 2][:], SIGMOID,
                scale=SIG_SCALE,
                accum_out=partials[:, 2 * t + 1:2 * t + 2])

    # ---------------- phase 1: block mask + counts ----------------
    # Software-pipelined emission: iteration t emits the chains for tile t,
    # the sigmoids for tile t-1 and the ACT subtract/indicator pair for
    # tile t+2 (two ahead, so it precedes the sigmoids in the ACT queue).
    xts = {}
    noise_dma(0)
    sub(0)
    noise_dma(1)
    sub(1)
    for t in range(NTILES):
        if t + 2 < NTILES:
            noise_dma(t + 2)
        if 1 <= t and t - 1 < X_PREFETCH:
            # x prefetches ride the sync queue one iteration behind the
            # noise stream: late noise tiles then queue behind fewer x
            # transfers, pulling the count (and the store stream) earlier.
            xp = t - 1
            xts[xp] = x_pool.tile([P, OPIX], F32, name=f"xt{rep}_{xp}",
                                  tag="xt")
            nc.sync.dma_start(xts[xp][:], x_ap[bass.ts(xp, P), :])
        mP = bufs["mP"][t % 2]
        m3 = mP[:].rearrange("p (h w) -> p h w", w=NW)
        tbD = bufs["tbD"][t % 2]
        tbD3 = tbD[:].rearrange("p (h w) -> p h w", w=NW)

        # ---- Pool chain: additive 5x5 window count of the indicators ----
        # a+[k] = m[k]+m[k+1]; c+[k] = a+[k]+m[k+2];
        # V[r] = sum of 5 rows = c+[r-4]+c+[r-2]; boundary rows 0..3 are
        # shorter windows (copies / one add).  fp8 sums never round to 0.
        nc.gpsimd.tensor_tensor(
            aP[:], mP[:, 0:(S - 2) * NW], mP[:, NW:(S - 1) * NW], ADD)
        nc.gpsimd.tensor_tensor(
            bP[:], aP[:], mP[:, 2 * NW:S * NW], ADD)
        nc.gpsimd.tensor_copy(vpP3[:, 0:1, BLK - 1:BLK - 1 + NW],
                              m3[:, 0:1, :])
        nc.gpsimd.tensor_tensor(
            vpP3[:, 4:S, BLK - 1:BLK - 1 + NW], bP3[:, 0:S - 4, :],
            bP3[:, 2:S - 2, :], ADD)
        nc.gpsimd.tensor_copy(vpP3[:, 1:2, BLK - 1:BLK - 1 + NW],
                              aP3[:, 0:1, :])
        nc.gpsimd.tensor_copy(vpP3[:, 2:3, BLK - 1:BLK - 1 + NW],
                              bP3[:, 0:1, :])
        nc.gpsimd.tensor_tensor(vpP3[:, 3:4, BLK - 1:BLK - 1 + NW],
                                aP3[:, 0:1, :], aP3[:, 2:3, :], ADD)
        # horizontal log-step into the compact a2P/b2P layouts
        nc.gpsimd.tensor_tensor(
            a2P3[:, :, :], vpP3[:, :, 0:VPW - 2], vpP3[:, :, 1:VPW - 1],
            ADD)
        nc.gpsimd.tensor_tensor(
            b2P3[:, :, :], a2P3[:, :, 0:W], a2P3[:, :, 2:VPW - 2], ADD)
        dmP3 = bufs["dmP"][t % 2][:].rearrange("p (h w) -> p h w", w=W)
        nc.gpsimd.tensor_tensor(
            dmP3[:, :, :], b2P3[:, :, :], vpP3[:, :, BLK - 1:VPW], ADD)

        # ---- DVE chain in bf16: min-pool for output rows [S, 56) ----
        nc.vector.tensor_tensor(
            aD[:, 0:(56 - S) * NW], tbD[:, 0:(56 - S) * NW],
            tbD[:, NW:(57 - S) * NW], MIN)
        nc.vector.tensor_tensor(
            btD[:], aD[:, 0:DR * NW], aD[:, 2 * NW:(58 - S) * NW], MIN)
        nc.vector.tensor_tensor(
            vpD3[:, :, BLK - 1:BLK - 1 + NW],
            btD[:].rearrange("p (h w) -> p h w", w=NW)[:, :, :],
            tbD3[:, BLK - 1:60 - S, :], MIN)
        nc.vector.tensor_tensor(
            a2D3[:, :, :], vpD3[:, :, 0:VPW - 2], vpD3[:, :, 1:VPW - 1],
            MIN)
        nc.vector.tensor_tensor(
            b2D3[:, :, :], a2D3[:, :, 0:W], a2D3[:, :, 2:VPW - 2], MIN)
        dmD3 = bufs["dmD"][t % 2][:].rearrange("p (h w) -> p h w", w=W)
        if t == NTILES - 1:
            # split the last tile's final min so its first-half sigmoid
            # overlaps the second half's compute.
            R1 = DR // 2
            nc.vector.tensor_tensor(
                dmD3[:, 0:R1, :], b2D3[:, 0:R1, :],
                vpD3[:, 0:R1, BLK - 1:VPW], MIN)
            nc.vector.tensor_tensor(
                dmD3[:, R1:DR, :], b2D3[:, R1:DR, :],
                vpD3[:, R1:DR, BLK - 1:VPW], MIN)
        else:
            nc.vector.tensor_tensor(
                dmD3[:, :, :], b2D3[:, :, :], vpD3[:, :, BLK - 1:VPW],
                MIN)

        if t >= 1:
            sigmoid(t - 1)
        if t + 2 < NTILES:
            sub(t + 2)
    sigmoid(NTILES - 1)

    # ------------- global count -> scale = M / count_ones -------------
    phead = bp.tile([P, 1], F32, name=f"phead{rep}", tag="phead")
    nc.vector.tensor_reduce(phead[:], partials[:, 0:2 * NTILES],
                            mybir.AxisListType.X, mybir.AluOpType.add)
    ptot = bp.tile([P, 1], F32, name=f"ptot{rep}", tag="ptot")
    nc.vector.tensor_tensor(ptot[:], phead[:],
                            partials[:, 2 * NTILES:2 * NTILES + 1],
                            mybir.AluOpType.add)
    pall = bp.tile([P, 1], F32, name=f"pall{rep}", tag="pall")
    nc.gpsimd.partition_all_reduce(pall[:], ptot[:], channels=P,
                                   reduce_op=bass_isa.ReduceOp.add)
    if single_core:
        tot_sb = pall
    else:
        cc_in = dram_pool.tile([P, 1], F32, name=f"cc_in{rep}", tag="cc_in")
        cc_out = dram_pool.tile([P, 1], F32, name=f"cc_out{rep}",
                                tag="cc_out")
        # scalar queue: the sync queue still holds pending x loads here,
        # which would delay the count round-trip by a full tile transfer.
        nc.scalar.dma_start(cc_in[:], pall[:])
        nc.gpsimd.collective_compute(
            "AllReduce", mybir.AluOpType.add,
            replica_groups=[list(range(N_CORES))],
            ins=[cc_in.opt()], outs=[cc_out.opt()])
        tot_sb = bp.tile([P, 1], F32, name=f"tot{rep}", tag="tot")
        nc.scalar.dma_start(tot_sb[:], cc_out[:])
    recip = bp.tile([P, 1], F32, name=f"recip{rep}", tag="recip")
    nc.vector.reciprocal(recip[:], tot_sb[:])
    scale_sb = bp.tile([P, 1], F32, name=f"scale{rep}", tag="scale")
    nc.vector.tensor_scalar_mul(scale_sb[:], recip[:], COUNT_M)

    # ---------------- phase 2: out = (x*scale)*mask ----------------
    # all multiplies on DVE (the Pool ucode has no mult); the store
    # stream (4.46us/tile of DMA) paces this loop anyway.
    for t in range(NTILES):
        if t in xts:
            xt = xts[t]
        else:
            xt = x_pool.tile([P, OPIX], F32, name=f"xt{rep}_{t}", tag="xt")
            nc.sync.dma_start(xt[:], x_ap[bass.ts(t, P), :])
        mslice = mask_store[:, t * OPIX:(t + 1) * OPIX]
        # the first two tiles' multiplies+stores are split (quarters then
        # halves) so the store stream ramps as early as possible after the
        # count lands; later tiles' multiplies finish well ahead of their
        # store slot.
        pieces = 4 if t == 0 else (2 if t <= 4 else 1)
        Q = OPIX // pieces
        for q in range(pieces):
            nc.vector.scalar_tensor_tensor(
                xt[:, q * Q:(q + 1) * Q], xt[:, q * Q:(q + 1) * Q],
                scale_sb[:, 0:1], mslice[:, q * Q:(q + 1) * Q],
                MULT, MULT)
            nc.scalar.dma_start(out_ap[bass.ts(t, P), q * Q:(q + 1) * Q],
                                xt[:, q * Q:(q + 1) * Q])


def _get_nc():
    if "nc" not in _CACHE:
        _CACHE["nc"] = _build()
    return _CACHE["nc"]


def kernel(x: np.ndarray, noise: np.ndarray) -> np.ndarray:
    x = np.asarray(x, dtype=np.float32)
    noise = np.asarray(noise, dtype=np.float32)
    assert x.shape == (B, C, H, W) and noise.shape == (B, C, NH, NW)
    nc = _get_nc()
    in_maps = []
    for i in range(N_CORES):
        xs = np.ascontiguousarray(x[i * B_SH:(i + 1) * B_SH]).reshape(
            IMGS, OPIX)
        ns = np.ascontiguousarray(noise[i * B_SH:(i + 1) * B_SH]).reshape(
            IMGS, NPIX)
        in_maps.append({"x": xs, "noise": ns})
    res = run_bass_kernel_spmd(nc, in_maps, list(range(N_CORES)))
    out = np.empty((B, C, H, W), dtype=np.float32)
    for i in range(N_CORES):
        out[i * B_SH:(i + 1) * B_SH] = res.results[i]["out"].reshape(
            B_SH, C, H, W)
    return out
